# revision 22
# baseline (speedup 1.0000x reference)
"""Trainium2 Bass kernel for a dense transformer block (B=2, T=2048, D=1024, H=16).

Sharding (8 NeuronCores, one chip):
  - Token-split for LN / residual / MLP: core i owns 512 tokens (rows 512i:512i+512
    of the flattened [4096, 1024] activation).
  - QKV is computed token-locally (each core: its 512 tokens x all 3072 qkv
    features), then ONE AllToAll ships per-head Q,K,V to head owners
    (core i owns heads {2i, 2i+1}).  V is transposed to token-major on the
    sender so the receiver can DMA it straight into the packed V buffer.
  - AllToAll of per-head attention outputs back to token owners.

Everything on-chip is feature-major ("transposed", [feature, token]).  LN1 uses
natural-layout DVE reductions with ln1_w folded into w_qkv and ln1_b folded
into a per-output-feature bias applied by the ACT engine; LN2 stats use PE
ones-matmuls with ln2_w folded into w_fc1.  Residuals ride as bf16
feature-major tiles injected into PSUM accumulators via identity matmuls.
"""

import numpy as np
import ml_dtypes

import concourse.bass as bass
import concourse.mybir as mybir
import concourse.tile as tile
from concourse import bacc
from concourse import bass_utils

F32 = mybir.dt.float32
MM_DT = mybir.dt.bfloat16
MM_NP = ml_dtypes.bfloat16

P = 128            # partitions
TB = 512           # tokens per core
D = 1024           # model dim
CT = D // P        # 8 feature tiles
NC = 8             # cores
TOKS = 2 * 2048    # global tokens
FF = 4096          # mlp hidden
DH = 64            # head dim
VW = 2 * (DH + 1)  # per-k-tile V columns: 2 heads x (64 dims + 1 ones col)

AF = mybir.ActivationFunctionType
OP = mybir.AluOpType
RG = [list(range(NC))]

_cache: dict = {}


def _build():
    nc = bacc.Bacc(
        "TRN2",
        target_bir_lowering=False,
        debug=False,
        enable_asserts=False,
        num_devices=NC,
    )

    # ---- kernel I/O ----
    x_own = nc.dram_tensor("x_own", [TB, D], F32, kind="ExternalInput").ap()
    wqkv = nc.dram_tensor("wqkv", [D, 3 * D], MM_DT, kind="ExternalInput").ap()
    qkvbias = nc.dram_tensor("qkvbias", [P, 3 * D // P], F32,
                             kind="ExternalInput").ap()
    wproj = nc.dram_tensor("wproj", [D, D], MM_DT, kind="ExternalInput").ap()
    wfc1 = nc.dram_tensor("wfc1", [D, FF], MM_DT, kind="ExternalInput").ap()
    wfc2 = nc.dram_tensor("wfc2", [FF, D], MM_DT, kind="ExternalInput").ap()
    id128m = nc.dram_tensor("id128m", [P, P], MM_DT, kind="ExternalInput").ap()
    cmask = nc.dram_tensor("cmask", [P, P], F32, kind="ExternalInput").ap()
    fc1B = nc.dram_tensor("fc1B", [P, FF // P], F32, kind="ExternalInput").ap()
    out_t = nc.dram_tensor("out_t", [D, TB], F32, kind="ExternalOutput").ap()

    HB = TB // 2   # 256 tokens per batch-stage per core

    with tile.TileContext(nc) as tc:
        with (
            tc.tile_pool(name="persist", bufs=1) as pers,
            tc.tile_pool(name="dram", bufs=1, space="DRAM") as dram,
        ):
            # collective staging (DRAM), one pair per batch-stage
            qkv_in = [dram.tile([NC, 3 * P, HB], MM_DT, name=f"qkv_in{b}",
                                tag=f"qkv_in{b}") for b in range(2)]
            qkv_out = [dram.tile([NC, 3 * P, HB], MM_DT, name=f"qkv_out{b}",
                                 tag=f"qkv_out{b}") for b in range(2)]
            a2a_in = [dram.tile([NC, P, HB], MM_DT, name=f"a2a_in{b}",
                                tag=f"a2a_in{b}") for b in range(2)]
            a2a_out = [dram.tile([NC, P, HB], MM_DT, name=f"a2a_out{b}",
                                 tag=f"a2a_out{b}") for b in range(2)]

            ones_col_m = pers.tile([P, 1], MM_DT, name="ones_col_m", tag="ones_col_m")
            nc.gpsimd.memset(ones_col_m[:], 1.0)
            ones_row_f = pers.tile([1, P], F32, name="ones_row_f", tag="ones_row_f")
            nc.gpsimd.memset(ones_row_f[:], 1.0)
            ones_row_m = pers.tile([1, P], MM_DT, name="ones_row_m", tag="ones_row_m")
            nc.gpsimd.memset(ones_row_m[:], 1.0)
            zb = pers.tile([P, 1], F32, name="zb", tag="zb")
            nc.gpsimd.memset(zb[:], 0.0)
            epsP = pers.tile([P, 1], F32, name="epsP", tag="epsP")
            nc.gpsimd.memset(epsP[:], 1e-5)
            eps1 = pers.tile([1, 1], F32, name="eps1", tag="eps1")
            nc.gpsimd.memset(eps1[:], 1e-5)
            fc1B_sb = pers.tile([P, FF // P], F32, name="fc1B_sb", tag="fc1B_sb")
            qkvb_sb = pers.tile([P, 3 * D // P], F32, name="qkvb_sb", tag="qkvb_sb")

            idm_sb = pers.tile([P, P], MM_DT, name="idm_sb", tag="idm_sb")
            nc.sync.dma_start(idm_sb[:], id128m[:])
            cmask_sb = pers.tile([P, P], F32, name="cmask_sb", tag="cmask_sb")
            nc.sync.dma_start(cmask_sb[:], cmask[:])

            # persistent activations (cols 0:256 = batch 0, 256:512 = batch 1)
            xTm = [pers.tile([P, TB], MM_DT, name=f"xTm{c}", tag=f"xTm{c}")
                   for c in range(CT)]
            x2Tm = [pers.tile([P, TB], MM_DT, name=f"x2Tm{c}", tag=f"x2Tm{c}")
                    for c in range(CT)]
            x2n = [pers.tile([P, TB], MM_DT, name=f"x2n{c}", tag=f"x2n{c}")
                   for c in range(CT)]
            g1 = [pers.tile([P, TB], MM_DT, name=f"g1_{h}", tag=f"g1_{h}")
                  for h in range(FF // P)]

            # ========== Phase A: x load, LN1, per-batch QKV -> 2x AllToAll ======
            with (
                tc.tile_pool(name="ps_t_a", bufs=4, space="PSUM") as ps_t_a,
                tc.tile_pool(name="ps_qkv", bufs=3, space="PSUM") as ps_qkv,
                tc.tile_pool(name="work_a", bufs=2) as work_a,
                tc.tile_pool(name="xn_a", bufs=1) as xn_a,
            ):
                wqkv_sb = [xn_a.tile([P, 3 * D], MM_DT, name=f"wqkv_sb{c}",
                                     tag=f"wqkv_sb{c}") for c in range(CT)]
                xrow_l = [xn_a.tile([P, D], F32, name=f"xrow{r}", tag=f"xrow{r}")
                          for r in range(TB // P)]
                xnn_l = [xn_a.tile([P, D], MM_DT, name=f"xnn{r}", tag=f"xnn{r}")
                         for r in range(TB // P)]
                xnT = [xn_a.tile([P, TB], MM_DT, name=f"xnT{c}", tag=f"xnT{c}")
                       for c in range(CT)]
                for r in range(TB // P):
                    nc.sync.dma_start(xrow_l[r][:], x_own[r * P:(r + 1) * P, :])
                nc.sync.dma_start(qkvb_sb[:], qkvbias[:])
                nc.sync.dma_start(fc1B_sb[:], fc1B[:])
                for c in range(CT):
                    nc.sync.dma_start(wqkv_sb[c][:], wqkv[c * P:(c + 1) * P, :])
                def ln_row(r):
                    xrow = xrow_l[r]
                    s1 = work_a.tile([P, 1], F32, name=f"s1a_{r}", tag="lns1")
                    nc.vector.tensor_reduce(s1[:], xrow[:],
                                            axis=mybir.AxisListType.X, op=OP.add)
                    sqf = work_a.tile([P, D], F32, name=f"sqa_{r}", tag="lnsq")
                    s2 = work_a.tile([P, 1], F32, name=f"s2a_{r}", tag="lns2")
                    nc.vector.scalar_tensor_tensor(
                        out=sqf[:], in0=xrow[:], scalar=1.0, in1=xrow[:],
                        op0=OP.mult, op1=OP.mult, accum_out=s2[:])
                    mu = work_a.tile([P, 1], F32, name=f"mua_{r}", tag="lnmu")
                    nc.vector.tensor_scalar_mul(mu[:], s1[:], 1.0 / D)
                    ex2 = work_a.tile([P, 1], F32, name=f"ex2a_{r}", tag="lnex2")
                    nc.vector.tensor_scalar_mul(ex2[:], s2[:], 1.0 / D)
                    mu2 = work_a.tile([P, 1], F32, name=f"mu2a_{r}", tag="lnmu2")
                    nc.vector.tensor_mul(mu2[:], mu[:], mu[:])
                    var = work_a.tile([P, 1], F32, name=f"vara_{r}", tag="lnvar")
                    nc.vector.tensor_sub(var[:], ex2[:], mu2[:])
                    sd = work_a.tile([P, 1], F32, name=f"sda_{r}", tag="lnsd")
                    nc.scalar.activation(sd[:], var[:], AF.Sqrt, bias=epsP[:])
                    rstd = work_a.tile([P, 1], F32, name=f"rstda_{r}", tag="lnrstd")
                    nc.vector.reciprocal(rstd[:], sd[:])
                    nc.vector.tensor_scalar(xnn_l[r][:], xrow[:], mu[:], rstd[:],
                                            OP.subtract, OP.mult)

                def transpose_rows(rows):
                    for r in rows:
                        for c in range(CT):
                            ptn = ps_t_a.tile([P, P], MM_DT, name=f"ptn_{r}_{c}",
                                              tag="ptn")
                            nc.tensor.transpose(ptn[:],
                                                xnn_l[r][:, c * P:(c + 1) * P],
                                                idm_sb[:])
                            nc.vector.tensor_copy(
                                xnT[c][:, r * P:(r + 1) * P], ptn[:])

                def qkv_mm_pass(b):
                    bc = slice(b * HB, (b + 1) * HB)
                    qgv = []
                    for d in range(NC):
                        for g in range(3):
                            fcol = (d * 3 + g) * P
                            bidx = d * 3 + g
                            ps = ps_qkv.tile([P, HB], F32, name=f"qkv{b}_{d}_{g}",
                                             tag="qkvps")
                            for c in range(CT):
                                nc.tensor.matmul(
                                    ps[:], wqkv_sb[c][:, fcol:fcol + P],
                                    xnT[c][:, bc],
                                    start=(c == 0), stop=(c == CT - 1))
                            qg = work_a.tile([P, HB], MM_DT, name=f"qg{b}_{d}_{g}",
                                             tag=("qkvm" if g < 2 else f"qkvv{d}"))
                            nc.scalar.activation(qg[:], ps[:], AF.Identity,
                                                 bias=qkvb_sb[:, bidx:bidx + 1])
                            if g < 2:
                                nc.sync.dma_start(
                                    qkv_in[b][d, g * P:(g + 1) * P, :], qg[:])
                            else:
                                qgv.append(qg)
                    return qgv

                def qkv_v_pass(b, qgv):
                    for d in range(NC):
                        vsend = work_a.tile([P, HB], MM_DT,
                                            name=f"vs{b}_{d}", tag="vsend")
                        for u in range(HB // P):
                            ptv = ps_t_a.tile([P, P], MM_DT,
                                              name=f"ptv{b}_{d}_{u}",
                                              tag="ptn")
                            nc.tensor.transpose(
                                ptv[:], qgv[d][:, u * P:(u + 1) * P],
                                idm_sb[:])
                            nc.vector.tensor_copy(
                                vsend[:, u * P:(u + 1) * P], ptv[:])
                        nc.sync.dma_start(
                            qkv_in[b][d, 2 * P:3 * P, :], vsend[:])
                    nc.gpsimd.collective_compute(
                        "AllToAll", OP.bypass, replica_groups=RG,
                        ins=[qkv_in[b][:]], outs=[qkv_out[b][:]])

                ln_row(0)
                ln_row(1)
                transpose_rows((0, 1))
                ln_row(2)
                ln_row(3)
                qgv0 = qkv_mm_pass(0)
                transpose_rows((2, 3))
                qkv_v_pass(0, qgv0)
                qgv1 = qkv_mm_pass(1)
                qkv_v_pass(1, qgv1)
                # residual-path transposes of raw x (bf16): fill the A2A wait
                for r in range(TB // P):
                    xm = work_a.tile([P, D], MM_DT, name=f"xma_{r}", tag="lnxm")
                    nc.vector.tensor_copy(xm[:], xrow_l[r][:])
                    for c in range(CT):
                        pt = ps_t_a.tile([P, P], MM_DT, name=f"pt_a_{r}_{c}",
                                         tag="ptn")
                        nc.tensor.transpose(pt[:], xm[:, c * P:(c + 1) * P],
                                            idm_sb[:])
                        nc.vector.tensor_copy(xTm[c][:, r * P:(r + 1) * P], pt[:])

            # ========== Phases B+C+D: per-batch attention pipeline ==============
            with (
                tc.tile_pool(name="wp_pool", bufs=1) as wp_pool,
                tc.tile_pool(name="w1_pool", bufs=1) as w1_pool,
            ):
                wproj_sb = []
                for dtl in range(CT):
                    tl = wp_pool.tile([P, D], MM_DT, name=f"wproj_sb{dtl}",
                                      tag=f"wproj_sb{dtl}")
                    nc.scalar.dma_start(tl[:], wproj[dtl * P:(dtl + 1) * P, :])
                    wproj_sb.append(tl)
                w1t = []
                for blk in range(FF // (4 * P)):
                    wt = w1_pool.tile([P, 4 * D], MM_DT, name=f"w1t{blk}",
                                      tag=f"w1t{blk}")
                    nc.scalar.dma_start(
                        wt[:].rearrange("p (c n) -> p c n", c=CT),
                        wfc1[:, blk * 4 * P:(blk + 1) * 4 * P]
                        .rearrange("(c p) n -> p c n", p=P))
                    w1t.append(wt)

                with (
                    tc.tile_pool(name="ps_aux", bufs=1, space="PSUM") as ps_aux,
                    tc.tile_pool(name="ps_s", bufs=4, space="PSUM") as ps_s_pool,
                    tc.tile_pool(name="ps_y", bufs=1, space="PSUM") as ps_y_pool,
                    tc.tile_pool(name="ps_mlp", bufs=1, space="PSUM") as ps_mlp,
                    tc.tile_pool(name="battn", bufs=1) as battn,
                    tc.tile_pool(name="work_b", bufs=3) as work_b,
                    tc.tile_pool(name="work_c", bufs=1) as work_c,
                    tc.tile_pool(name="epool", bufs=6) as epool,
                ):
                    mlpbank = ps_mlp.tile([P, TB], F32, name="mlpbank",
                                          tag="mlps")
                    qT = battn.tile([P, TOKS], MM_DT, name="qT", tag="qT")
                    kT = battn.tile([P, TOKS], MM_DT, name="kT", tag="kT")
                    V_sb = battn.tile([P, 32 * VW], MM_DT, name="V_sb", tag="V_sb")
                    nc.gpsimd.memset(V_sb[:], 1.0)
                    yall = [battn.tile([P, NC * HB], MM_DT, name=f"yall{b}",
                                       tag="yall") for b in range(2)]

                    def stage_loads(b):
                        bq = slice(b * 2048, (b + 1) * 2048)
                        nc.sync.dma_start(
                            qT[:, bq].rearrange("p (i t) -> p i t", i=NC),
                            qkv_out[b][:, 0:P, :].rearrange("i p t -> p i t"))
                        nc.sync.dma_start(
                            kT[:, bq].rearrange("p (i t) -> p i t", i=NC),
                            qkv_out[b][:, P:2 * P, :].rearrange("i p t -> p i t"))
                        for i in range(NC):
                            gk0 = b * 16 + 2 * i
                            nc.sync.dma_start(
                                V_sb[:, gk0 * VW:gk0 * VW + 4 * (DH + 1)]
                                .rearrange("p (uh f) -> p uh f", uh=4, f=DH + 1)
                                [:, :, 0:DH],
                                qkv_out[b][i, 2 * P:3 * P, :]
                                .rearrange("p (uh f) -> p uh f", uh=4, f=DH))

                    def attn_block(b, j):
                        qoff = b * 2048 + j * TB
                        ps_y = [
                            ps_y_pool.tile([DH + 1, TB], F32,
                                           name=f"psy{b}_{j}_{h}", tag=f"psy{h}")
                            for h in range(2)
                        ]
                        nkt = 4 * j + 4
                        for kt in range(nkt):
                            gk = b * 16 + kt
                            n0 = 0 if kt < 4 * j else (kt - 4 * j) * P
                            for h in range(2):
                                pss = ps_s_pool.tile(
                                    [P, TB], F32,
                                    name=f"pss{b}_{j}_{kt}_{h}", tag="pss")
                                nc.tensor.matmul(
                                    pss[:, n0:TB],
                                    kT[h * DH:(h + 1) * DH, gk * P:(gk + 1) * P],
                                    qT[h * DH:(h + 1) * DH, qoff + n0: qoff + TB],
                                    start=True, stop=True,
                                    tile_position=(h * DH, 0))
                                if kt >= 4 * j:
                                    nc.vector.tensor_add(
                                        pss[:, n0:n0 + P], pss[:, n0:n0 + P],
                                        cmask_sb[:])
                                et = epool.tile([P, TB], MM_DT,
                                                name=f"et{b}_{j}_{kt}_{h}",
                                                tag=f"et{h}")
                                vsl = V_sb[:, gk * VW + h * (DH + 1):
                                           gk * VW + h * (DH + 1) + DH + 1]
                                nc.scalar.activation(et[:, n0:TB], pss[:, n0:TB],
                                                     AF.Exp, bias=zb[:])
                                nc.tensor.matmul(
                                    ps_y[h][:, n0:TB], vsl, et[:, n0:TB],
                                    start=(kt == 0), stop=(kt == nkt - 1))
                        yn = work_b.tile([P, TB], MM_DT, name=f"yn{b}_{j}",
                                         tag="yn")
                        for h in range(2):
                            rf = work_b.tile([1, TB], F32, name=f"rf{b}_{j}_{h}",
                                             tag="rf")
                            nc.vector.reciprocal(rf[:], ps_y[h][DH:DH + 1, :])
                            rm = work_b.tile([1, TB], MM_DT, name=f"rm{b}_{j}_{h}",
                                             tag="rm")
                            nc.vector.tensor_copy(rm[:], rf[:])
                            ps_rb = ps_aux.tile([DH, TB], F32,
                                                name=f"psrb{b}_{j}_{h}", tag="aux")
                            nc.tensor.matmul(ps_rb[:], ones_row_m[:, 0:DH], rm[:],
                                             start=True, stop=True)
                            rb = work_b.tile([DH, TB], MM_DT,
                                             name=f"rb{b}_{j}_{h}", tag="rb")
                            nc.vector.tensor_copy(rb[:], ps_rb[:])
                            nc.vector.tensor_mul(
                                yn[h * DH:(h + 1) * DH, :],
                                ps_y[h][0:DH, :], rb[:])
                        # scatter to the 2 token-owner slots of this j-block
                        nc.sync.dma_start(
                            a2a_in[b][2 * j:2 * j + 2]
                            .rearrange("d p t -> p d t"),
                            yn[:].rearrange("p (d t) -> p d t", d=2))

                    def proj_ln2_stage(b):
                        bc = slice(b * HB, (b + 1) * HB)
                        nc.sync.dma_start(
                            yall[b][:].rearrange("p (d t) -> p d t", d=NC),
                            a2a_out[b][:].rearrange("d p t -> p d t"))
                        lns = ps_aux.tile([DH, TB], F32, name=f"lns{b}",
                                          tag="aux")
                        for c in range(CT):
                            ps = mlpbank[:, (c % 2) * HB:(c % 2 + 1) * HB]
                            for dtl in range(NC):
                                nc.tensor.matmul(
                                    ps, wproj_sb[dtl][:, c * P:(c + 1) * P],
                                    yall[b][:, dtl * HB:(dtl + 1) * HB],
                                    start=(dtl == 0), stop=False)
                            nc.tensor.matmul(ps, idm_sb[:], xTm[c][:, bc],
                                             start=False, stop=True)
                            nc.vector.tensor_copy(x2Tm[c][:, bc], ps)
                            nc.tensor.matmul(lns[0:1, 0:HB], ones_col_m[:],
                                             x2Tm[c][:, bc],
                                             start=(c == 0), stop=(c == CT - 1))
                            sq = work_c.tile([P, HB], MM_DT, name=f"sq{b}_{c}",
                                             tag="ln_sq")
                            nc.vector.tensor_mul(sq[:], x2Tm[c][:, bc],
                                                 x2Tm[c][:, bc])
                            nc.tensor.matmul(lns[32:33, 0:HB], ones_col_m[:],
                                             sq[:],
                                             start=(c == 0), stop=(c == CT - 1),
                                             tile_position=(0, 32))
                        mu = work_c.tile([1, HB], F32, name=f"mu{b}", tag="ln_mu")
                        nc.vector.tensor_scalar_mul(mu[:], lns[0:1, 0:HB], 1.0 / D)
                        mu2 = work_c.tile([1, HB], F32, name=f"mu2{b}",
                                          tag="ln_mu2")
                        nc.vector.tensor_mul(mu2[:], mu[:], mu[:])
                        vr = work_c.tile([1, HB], F32, name=f"vr{b}", tag="ln_vr")
                        nc.vector.scalar_tensor_tensor(
                            out=vr[:], in0=lns[32:33, 0:HB], scalar=1.0 / D,
                            in1=mu2[:], op0=OP.mult, op1=OP.subtract)
                        sd = work_c.tile([1, HB], F32, name=f"sd{b}", tag="ln_sd")
                        nc.scalar.activation(sd[:], vr[:], AF.Sqrt, bias=eps1[:])
                        rs = work_c.tile([1, HB], F32, name=f"rs{b}", tag="ln_rs")
                        nc.vector.reciprocal(rs[:], sd[:])
                        mrs = work_c.tile([1, HB], F32, name=f"mrs{b}",
                                          tag="ln_mrs")
                        nc.vector.tensor_mul(mrs[:], mu[:], rs[:])
                        ps_rs = mlpbank[:, 0:HB]
                        nc.tensor.matmul(ps_rs, ones_row_f[:], rs[:],
                                         start=True, stop=True)
                        rs_b = work_c.tile([P, HB], F32, name=f"rsb{b}",
                                           tag="rs_b")
                        nc.vector.tensor_copy(rs_b[:], ps_rs)
                        ps_mrs = mlpbank[:, HB:TB]
                        nc.tensor.matmul(ps_mrs, ones_row_f[:], mrs[:],
                                         start=True, stop=True)
                        mrs_b = work_c.tile([P, HB], F32, name=f"mrsb{b}",
                                            tag="mrs_b")
                        nc.vector.tensor_copy(mrs_b[:], ps_mrs)
                        for c in range(CT):
                            xr = work_c.tile([P, HB], F32, name=f"xr{b}_{c}",
                                             tag="xr")
                            nc.vector.tensor_mul(xr[:], x2Tm[c][:, bc], rs_b[:])
                            nc.vector.tensor_sub(x2n[c][:, bc], xr[:], mrs_b[:])

                    def fc1_stage(b):
                        bc = slice(b * HB, (b + 1) * HB)
                        for ht in range(FF // P):
                            blk, hh = ht // 4, ht % 4
                            ps = mlpbank[:, (ht % 2) * HB:(ht % 2 + 1) * HB]
                            for c in range(CT):
                                nc.tensor.matmul(
                                    ps,
                                    w1t[blk][:, (c * 4 + hh) * P:
                                             (c * 4 + hh + 1) * P],
                                    x2n[c][:, bc],
                                    start=(c == 0), stop=(c == CT - 1))
                            nc.scalar.activation(g1[ht][:, bc], ps, AF.Gelu,
                                                 bias=fc1B_sb[:, ht:ht + 1])

                    # ---- emission schedule ----
                    stage_loads(0)
                    stage_loads(1)
                    for j in range(4):
                        attn_block(0, j)
                    nc.gpsimd.collective_compute(
                        "AllToAll", OP.bypass, replica_groups=RG,
                        ins=[a2a_in[0][:]], outs=[a2a_out[0][:]])
                    for j in range(4):
                        attn_block(1, j)
                    nc.gpsimd.collective_compute(
                        "AllToAll", OP.bypass, replica_groups=RG,
                        ins=[a2a_in[1][:]], outs=[a2a_out[1][:]])
                    proj_ln2_stage(0)
                    fc1_stage(0)
                    proj_ln2_stage(1)
                    fc1_stage(1)

            # ================= Phase E: fc2 + residual -> out ===================
            with (
                tc.tile_pool(name="ps_fc2", bufs=1, space="PSUM") as ps_fc2,
                tc.tile_pool(name="w2pool", bufs=3) as w2pool,
                tc.tile_pool(name="x3pool", bufs=2) as x3pool,
            ):
                ps_m = [ps_fc2.tile([P, TB], F32, name=f"fc2_{c}", tag=f"fc2_{c}")
                        for c in range(CT)]
                for wb in range(8):
                    w2t = w2pool.tile([P, 4 * D], MM_DT, name=f"w2t{wb}",
                                      tag="w2t")
                    nc.scalar.dma_start(
                        w2t[:].rearrange("p (q n) -> p q n", q=4),
                        wfc2[wb * 4 * P:(wb + 1) * 4 * P, :]
                        .rearrange("(q p) n -> p q n", p=P))
                    for q in range(4):
                        ht = wb * 4 + q
                        for c in range(CT):
                            nc.tensor.matmul(
                                ps_m[c][:],
                                w2t[:, q * D + c * P:q * D + (c + 1) * P],
                                g1[ht][:],
                                start=(ht == 0), stop=False)
                for c in range(CT):
                    nc.tensor.matmul(ps_m[c][:], idm_sb[:], x2Tm[c][:],
                                     start=False, stop=True)
                    x3 = x3pool.tile([P, TB], F32, name=f"x3_{c}", tag="x3")
                    nc.vector.tensor_copy(x3[:], ps_m[c][:])
                    nc.sync.dma_start(out_t[c * P:(c + 1) * P, :], x3[:])

    nc.compile()
    return nc


def _prep_inputs(x, ln1_w, ln1_b, w_qkv, w_proj, ln2_w, ln2_b, w_fc1, w_fc2):
    xf = np.ascontiguousarray(np.asarray(x, np.float32).reshape(TOKS, D))
    ln1w_f = np.asarray(ln1_w, np.float32)
    ln1b_f = np.asarray(ln1_b, np.float32)
    wq = np.asarray(w_qkv[:, :D], np.float32) * 0.125  # fold 1/sqrt(dh)
    wk = np.asarray(w_qkv[:, D:2 * D], np.float32)
    wv = np.asarray(w_qkv[:, 2 * D:], np.float32)
    # pack per-dest: [q_d | k_d | v_d] blocks of 128 cols; fold ln1_w into rows
    packed = np.empty((D, 3 * D), np.float32)
    for d in range(NC):
        hs = slice(P * d, P * (d + 1))
        packed[:, 3 * P * d:3 * P * d + P] = wq[:, hs]
        packed[:, 3 * P * d + P:3 * P * d + 2 * P] = wk[:, hs]
        packed[:, 3 * P * d + 2 * P:3 * P * d + 3 * P] = wv[:, hs]
    packed_w = ln1w_f[:, None] * packed
    wqkv_p = packed_w.astype(MM_NP)
    # qkv bias = ln1_b @ packed (exact vs the bf16-rounded weights is fine)
    qb = ln1b_f @ packed
    qkvbias_np = np.ascontiguousarray(qb.reshape(3 * D // P, P).T)
    wproj = np.asarray(w_proj, np.float32).astype(MM_NP)
    wfc2 = np.asarray(w_fc2, np.float32).astype(MM_NP)
    idm = np.eye(P, dtype=np.float32).astype(MM_NP)
    pp, jj = np.meshgrid(np.arange(P), np.arange(P), indexing="ij")
    cm = np.where(pp <= jj, 0.0, -1e30).astype(np.float32)
    # LN2 w folded into fc1; B = ln2_b @ w_fc1 becomes the gelu bias
    w1p = (np.asarray(ln2_w, np.float32)[:, None] *
           np.asarray(w_fc1, np.float32))
    wfc1 = w1p.astype(MM_NP)
    Bv = np.asarray(ln2_b, np.float32) @ np.asarray(w_fc1, np.float32)
    fc1B_np = np.ascontiguousarray(Bv.reshape(FF // P, P).T)
    common = {
        "wqkv": wqkv_p, "qkvbias": qkvbias_np, "wproj": wproj,
        "wfc1": wfc1, "wfc2": wfc2, "fc1B": fc1B_np,
        "id128m": idm, "cmask": cm,
    }
    in_maps = []
    HB = TB // 2
    for i in range(NC):
        m = dict(common)
        m["x_own"] = np.ascontiguousarray(np.concatenate(
            [xf[HB * i:HB * (i + 1)], xf[2048 + HB * i:2048 + HB * (i + 1)]],
            axis=0))
        in_maps.append(m)
    return in_maps


def _get_runner():
    """Build (once) a cached, non-donating PJRT executable for the kernel."""
    if "runner" in _cache:
        return _cache["runner"]
    import jax
    from jax.sharding import Mesh, PartitionSpec, NamedSharding
    from jax.experimental.shard_map import shard_map
    from concourse import bass2jax

    nc = _cache.get("nc")
    if nc is None:
        nc = _cache["nc"] = _build()
    bass2jax.install_neuronx_cc_hook()
    partition_name = nc.partition_id_tensor.name if nc.partition_id_tensor else None
    in_names, out_names, out_avals, zero_outs = [], [], [], []
    for alloc in nc.m.functions[0].allocations:
        if not isinstance(alloc, mybir.MemoryLocationSet):
            continue
        name = alloc.memorylocations[0].name
        if alloc.kind == "ExternalInput":
            if name != partition_name:
                in_names.append(name)
        elif alloc.kind == "ExternalOutput":
            out_names.append(name)
            shape = tuple(alloc.tensor_shape)
            dtype = mybir.dt.np(alloc.dtype)
            out_avals.append(jax.core.ShapedArray(shape, dtype))
            zero_outs.append(np.zeros(shape, dtype))
    n_params = len(in_names)
    all_in_names = in_names + out_names + ([partition_name] if partition_name else [])

    def _body(*args):
        operands = list(args)
        if partition_name is not None:
            operands.append(bass2jax.partition_id_tensor())
        outs = bass2jax._bass_exec_p.bind(
            *operands, out_avals=tuple(out_avals), in_names=tuple(all_in_names),
            out_names=tuple(out_names), lowering_input_output_aliases=(),
            sim_require_finite=True, sim_require_nnan=True, nc=nc)
        return tuple(outs)

    devices = jax.devices()[:NC]
    mesh = Mesh(np.asarray(devices), ("core",))
    nin = n_params + len(out_names)
    sharded = jax.jit(shard_map(
        _body, mesh=mesh, in_specs=(PartitionSpec("core"),) * nin,
        out_specs=(PartitionSpec("core"),) * len(out_names), check_rep=False))
    sh = NamedSharding(mesh, PartitionSpec("core"))
    dev_zeros = [
        jax.device_put(np.zeros((NC * z.shape[0], *z.shape[1:]), z.dtype), sh)
        for z in zero_outs
    ]
    runner = (sharded, in_names, out_names, out_avals, sh, dev_zeros)
    _cache["runner"] = runner
    return runner


def kernel(**inputs):
    import jax
    sharded, in_names, out_names, out_avals, sh, dev_zeros = _get_runner()
    in_maps = _prep_inputs(**inputs)
    concat_in = [np.concatenate([in_maps[c][nm] for c in range(NC)], axis=0)
                 for nm in in_names]
    dev_in = [jax.device_put(a, sh) for a in concat_in]
    out_arrs = sharded(*dev_in, *dev_zeros)
    got = {nm: np.asarray(out_arrs[i]).reshape(NC, *out_avals[i].shape)
           for i, nm in enumerate(out_names)}
    out = np.empty((TOKS, D), np.float32)
    HB = TB // 2
    for i in range(NC):
        out[HB * i:HB * (i + 1)] = got["out_t"][i][:, 0:HB].T
        out[2048 + HB * i:2048 + HB * (i + 1)] = got["out_t"][i][:, HB:TB].T
    return out.reshape(2, 2048, D)


if __name__ == "__main__":
    rng = np.random.default_rng(0)
    ins = {
        "x": rng.standard_normal((2, 2048, D), dtype=np.float32),
        "ln1_w": np.ones(D, np.float32),
        "ln1_b": np.zeros(D, np.float32),
        "w_qkv": (rng.standard_normal((D, 3 * D), dtype=np.float32) / 32.0),
        "w_proj": (rng.standard_normal((D, D), dtype=np.float32) / 32.0),
        "ln2_w": np.ones(D, np.float32),
        "ln2_b": np.zeros(D, np.float32),
        "w_fc1": (rng.standard_normal((D, FF), dtype=np.float32) / 32.0),
        "w_fc2": (rng.standard_normal((FF, D), dtype=np.float32) / 64.0),
    }
    out = kernel(**ins)
    print("kernel out", out.shape, out.dtype, float(np.abs(out).mean()))
